# revision 1
# baseline (speedup 1.0000x reference)
"""Trainium2 Bass kernel for nn_ContentAgree (dense transformer block).

Sharding: 8 cores = 4 batches x 2 sequence-halves. Each core processes
8192 tokens of one batch through the full block. The only cross-core
dependency is the attn3 softmax over the full sequence (N=16384) and
w2 = attn3 @ V: cores compute exp-sum partials + numerator partials and
a pairwise AllReduce combines the two halves of each batch.

Layout conventions per core (partition dim first):
  xsT   (256 ch, 8192 tok)  "T-layout" - directly matches x[b] in DRAM
  QT    (256, T)  rows h*64+hd
  V     (T, 256)  token-major ("N-layout"), cols h*64+hd
  scoresT/pT (128 = [k of head pair], T)
  p_N   (T, 256)  cols k of 4 heads
  w1uT  (128 = [k' of head pair], 8192)   persists in SBUF
  out/ffn tiles (ch, T) with channels PERMUTED to (h,hd)-major; the
  reference merges heads as channel = hd*4+h, handled by permuted reads
  of x/W1/W2/ln vectors and a permuted final store (pure AP tricks).
"""
import sys

for _p in ("/root/.axon_site/_ro/trn_rl_repo", "/opt/trn_rl_repo"):
    if _p not in sys.path:
        sys.path.append(_p)

import numpy as np
import concourse.bass as bass
import concourse.bacc as bacc
import concourse.tile as tile
from concourse import mybir
from concourse.bass_utils import run_bass_kernel_spmd

dt = mybir.dt
F32 = dt.float32
F32R = dt.float32r
AF = mybir.ActivationFunctionType
ALU = mybir.AluOpType
AX = mybir.AxisListType

P = 128
T = 512                 # token tile width (free dim)
NT = 8192               # tokens per core
NTILES = NT // T        # 16
NSUB = T // P           # 4
DIMV = 256
KK = 64
SCALE = 0.125           # 1/sqrt(64)
EPS = 1e-5

TRACE = False
TRACE_KW = {}


USE_F32R = False


def r(ap):
    return ap.bitcast(F32R) if USE_F32R else ap


def build_program():
    nc = bacc.Bacc("TRN2", target_bir_lowering=False, debug=False, num_devices=8)

    # ---------------- DRAM I/O ----------------
    xsT_d = nc.dram_tensor("xsT", [DIMV, NT], F32, kind="ExternalInput")
    cb_d = nc.dram_tensor("Cb", [KK, DIMV], F32, kind="ExternalInput")
    wq_d = nc.dram_tensor("Wq", [DIMV, DIMV], F32, kind="ExternalInput")
    wk_d = nc.dram_tensor("Wk", [DIMV, DIMV], F32, kind="ExternalInput")
    wv_d = nc.dram_tensor("Wv", [DIMV, DIMV], F32, kind="ExternalInput")
    bq_d = nc.dram_tensor("bq", [DIMV], F32, kind="ExternalInput")
    bk_d = nc.dram_tensor("bk", [DIMV], F32, kind="ExternalInput")
    bv_d = nc.dram_tensor("bv", [DIMV], F32, kind="ExternalInput")
    w1_d = nc.dram_tensor("W1", [DIMV, 2 * DIMV], F32, kind="ExternalInput")
    b1_d = nc.dram_tensor("b1", [2 * DIMV], F32, kind="ExternalInput")
    w2_d = nc.dram_tensor("W2", [2 * DIMV, DIMV], F32, kind="ExternalInput")
    b2_d = nc.dram_tensor("b2", [DIMV], F32, kind="ExternalInput")
    g1_d = nc.dram_tensor("ln1_g", [DIMV], F32, kind="ExternalInput")
    be1_d = nc.dram_tensor("ln1_b", [DIMV], F32, kind="ExternalInput")
    g2_d = nc.dram_tensor("ln2_g", [DIMV], F32, kind="ExternalInput")
    be2_d = nc.dram_tensor("ln2_b", [DIMV], F32, kind="ExternalInput")
    ident_d = nc.dram_tensor("ident", [P, P], F32, kind="ExternalInput")
    onesr_d = nc.dram_tensor("onesr", [1, P], F32, kind="ExternalInput")
    mcol_d = nc.dram_tensor("mcol", [P, 1], F32, kind="ExternalInput")
    esel_d = nc.dram_tensor("esel", [4, DIMV], F32, kind="ExternalInput")
    y_d = nc.dram_tensor("y", [DIMV, NT], F32, kind="ExternalOutput")


    # permuted DRAM views ((hd,h)-major channel -> (h,hd)-major), exposed as
    # 3-D APs (h, hd, ...) since rearrange cannot regroup non-adjacent dims;
    # DMA flattens nested dims, so a [2,64,T] view fills a [128,T] tile.
    xsP_v = xsT_d[:].rearrange("(hd h) n -> h hd n", h=4)
    w1P_v = w1_d[:].rearrange("(hd h) f -> h hd f", h=4)
    w2P_v = w2_d[:].rearrange("f (hd h) -> f h hd", h=4)
    yP_v = y_d[:].rearrange("(hd h) n -> h hd n", h=4)

    with tile.TileContext(nc) as tc:
        cpool = tc.tile_pool(name="consts", bufs=1)
        ppool = tc.tile_pool(name="persist", bufs=1)
        w1upool = tc.tile_pool(name="w1u", bufs=2 * NTILES)
        dpool = tc.tile_pool(name="drambounce", bufs=6, space="DRAM")
        with cpool as cp, ppool as pp, w1upool as wp, dpool as dp:
            # ---------------- load constants ----------------
            wk_sb = [cp.tile([P, DIMV], F32, name=f"wk{c}", tag=f"wk{c}") for c in range(2)]
            wv_sb = [cp.tile([P, DIMV], F32, name=f"wv{c}", tag=f"wv{c}") for c in range(2)]
            wq_sb = [cp.tile([P, DIMV], F32, name=f"wq{c}", tag=f"wq{c}") for c in range(2)]
            w1_sb = [cp.tile([P, 2 * DIMV], F32, name=f"w1{c}", tag=f"w1{c}") for c in range(2)]
            w2_sb = [cp.tile([P, DIMV], F32, name=f"w2{c}", tag=f"w2{c}") for c in range(4)]
            for c in range(2):
                nc.sync.dma_start(wk_sb[c][:], wk_d[c * P:(c + 1) * P, :])
                nc.sync.dma_start(wv_sb[c][:], wv_d[c * P:(c + 1) * P, :])
                nc.sync.dma_start(wq_sb[c][:], wq_d[c * P:(c + 1) * P, :])
                for j in range(2):
                    nc.sync.dma_start(w1_sb[c][j * KK:(j + 1) * KK, :],
                                      w1P_v[2 * c + j, :, :])
            for c in range(4):
                for j in range(4):
                    nc.sync.dma_start(w2_sb[c][:, j * KK:(j + 1) * KK],
                                      w2P_v[c * P:(c + 1) * P, j, :])

            bq_sb = cp.tile([P, 2], F32, name="bq", tag="bq")
            bk_sb = cp.tile([P, 2], F32, name="bk", tag="bk")
            b1_sb = cp.tile([P, 4], F32, name="b1", tag="b1")
            b2_sb = cp.tile([P, 2], F32, name="b2", tag="b2")
            g1_sb = cp.tile([P, 2], F32, name="g1", tag="g1")
            be1_sb = cp.tile([P, 2], F32, name="be1", tag="be1")
            g2_sb = cp.tile([P, 2], F32, name="g2", tag="g2")
            be2_sb = cp.tile([P, 2], F32, name="be2", tag="be2")
            nc.sync.dma_start(bq_sb[:], bq_d[:].rearrange("(c p) -> p c", p=P))
            nc.sync.dma_start(bk_sb[:], bk_d[:].rearrange("(c p) -> p c", p=P))
            nc.sync.dma_start(b1_sb[:], b1_d[:].rearrange("(m p) -> p m", p=P))
            for vd, vt in ((b2_d, b2_sb), (g1_d, g1_sb), (be1_d, be1_sb),
                           (g2_d, g2_sb), (be2_d, be2_sb)):
                vperm = vd[:].rearrange("(hd h) -> h hd", h=4)
                for c in range(2):
                    for j in range(2):
                        nc.sync.dma_start(vt[j * KK:(j + 1) * KK, c:c + 1],
                                          vperm[2 * c + j, :])

            bv_row = cp.tile([1, DIMV], F32, name="bvrow", tag="bvrow")
            nc.sync.dma_start(bv_row[:], bv_d[:].rearrange("(o d) -> o d", o=1))
            epsc = cp.tile([P, 1], F32, name="epsc", tag="epsc")
            nc.vector.memset(epsc[:], EPS)
            ident = cp.tile([P, P], F32, name="ident", tag="ident")
            onesr = cp.tile([1, P], F32, name="onesr", tag="onesr")
            mcol = cp.tile([P, 1], F32, name="mcol", tag="mcol")
            esel = cp.tile([4, DIMV], F32, name="esel", tag="esel")
            nc.sync.dma_start(ident[:], ident_d[:])
            nc.sync.dma_start(onesr[:], onesr_d[:])
            nc.sync.dma_start(mcol[:], mcol_d[:])
            nc.sync.dma_start(esel[:], esel_d[:])

            # ---------------- K1T / attn2 prep ----------------
            psA_cm = tc.tile_pool(name="psA", bufs=1, space="PSUM")
            psA = psA_cm.__enter__()
            psp = psA
            cb_sb = pp.tile([KK, DIMV], F32, name="cb", tag="cb")
            nc.sync.dma_start(cb_sb[:], cb_d[:])
            cbT0 = pp.tile([P, KK], F32, name="cbT0", tag="cbT0")
            cbT1 = pp.tile([P, KK], F32, name="cbT1", tag="cbT1")
            for c, cbTt in enumerate((cbT0, cbT1)):
                tp = psp.tile([P, KK], F32, name="cbT_ps", tag="cbT_ps")
                nc.tensor.matmul(tp[:], cb_sb[:, c * P:(c + 1) * P],
                                 ident[0:KK, 0:KK], is_transpose=True)
                nc.scalar.copy(cbTt[:], tp[:])

            k1t = [pp.tile([P, KK], F32, name=f"k1t{m}", tag=f"k1t{m}") for m in range(2)]
            for m in range(2):
                k1ps = psp.tile([P, KK], F32, name="k1_ps", tag="k1_ps")
                for c, cbTt in enumerate((cbT0, cbT1)):
                    nc.tensor.matmul(k1ps[:], r(wq_sb[c][:, m * P:(m + 1) * P]),
                                     r(cbTt[:]), start=(c == 0), stop=(c == 1))
                nc.scalar.activation(k1t[m][:], k1ps[:], AF.Identity,
                                     bias=bq_sb[:, m:m + 1])

            k1blk = [pp.tile([P, P], F32, name=f"k1blk{pr}", tag=f"k1blk{pr}") for pr in range(2)]
            for pr in range(2):
                nc.vector.memset(k1blk[pr][:], 0.0)
                nc.scalar.copy(k1blk[pr][0:KK, 0:KK], k1t[pr][0:KK, :])
                nc.scalar.copy(k1blk[pr][KK:P, KK:P], k1t[pr][KK:P, :])

            a2blk = [pp.tile([P, P], F32, name=f"a2blk{pr}", tag=f"a2blk{pr}") for pr in range(2)]
            for pr in range(2):
                scps = psp.tile([P, P], F32, name="a2_ps", tag="a2_ps")
                nc.tensor.matmul(scps[:], k1blk[pr][:], k1blk[pr][:])
                nc.vector.memset(a2blk[pr][:], 0.0)
                for hb in range(2):
                    sl = slice(hb * KK, (hb + 1) * KK)
                    mx = pp.tile([P, 1], F32, name="a2mx", tag="a2mx")
                    nc.vector.tensor_reduce(mx[sl, :], scps[sl, sl], AX.X, ALU.max)
                    nmx = pp.tile([P, 1], F32, name="a2nmx", tag="a2nmx")
                    nc.vector.tensor_scalar_mul(nmx[sl, :], mx[sl, :], -SCALE)
                    rsum = pp.tile([P, 1], F32, name="a2rs", tag="a2rs")
                    nc.scalar.activation(a2blk[pr][sl, sl], scps[sl, sl], AF.Exp,
                                         bias=nmx[sl, :], scale=SCALE,
                                         accum_out=rsum[sl, :])
                    rinv = pp.tile([P, 1], F32, name="a2ri", tag="a2ri")
                    nc.vector.reciprocal(rinv[sl, :], rsum[sl, :])
                    nc.vector.tensor_scalar(a2blk[pr][sl, sl], a2blk[pr][sl, sl],
                                            rinv[sl, :], None, ALU.mult)

            # persistent accumulators
            rs_N = pp.tile([P, 256], F32, name="rsN", tag="rsN")
            scol = [pp.tile([P, NTILES], F32, name=f"scol{pr}", tag=f"scol{pr}") for pr in range(2)]
            w1uT = [[None] * 2 for _ in range(NTILES)]

            # DRAM bounce tiles for the collective
            partial = dp.tile([257, 256], F32, name="partial", tag="partial")
            reduced = dp.tile([257, 256], F32, name="reduced", tag="reduced")

            psA_cm.__exit__(None, None, None)

            # ---------------- loop 1 ----------------
            with tc.tile_pool(name="l1", bufs=3) as l1, \
                 tc.tile_pool(name="l1ps", bufs=1, space="PSUM") as l1ps, \
                 tc.tile_pool(name="w2nps", bufs=1, space="PSUM") as w2nps:
                w2n_ps = [w2nps.tile([P, 256], F32, name=f"w2n{pr}", tag=f"w2n{pr}")
                          for pr in range(2)]
                for ti in range(NTILES):
                    tsl = slice(ti * T, (ti + 1) * T)
                    xs = [l1.tile([P, T], F32, name=f"xs{c}", tag=f"xs{c}") for c in range(2)]
                    for c in range(2):
                        nc.sync.dma_start(xs[c][:], xsT_d[c * P:(c + 1) * P, tsl])

                    qt = [l1.tile([P, T], F32, name=f"qt{m}", tag=f"qt{m}") for m in range(2)]
                    for m in range(2):
                        qtps = l1ps.tile([P, T], F32, name="qt_ps", tag="qt_ps")
                        for c in range(2):
                            nc.tensor.matmul(qtps[:],
                                             r(wk_sb[c][:, m * P:(m + 1) * P]),
                                             r(xs[c][:]),
                                             start=(c == 0), stop=(c == 1))
                        nc.scalar.activation(qt[m][:], qtps[:], AF.Identity,
                                             bias=bk_sb[:, m:m + 1])

                    # V (token-major) per 128-token subtile
                    vsb = [l1.tile([P, 256], F32, name=f"v{su}", tag=f"v{su}")
                           for su in range(NSUB)]
                    for su in range(NSUB):
                        ssl = slice(su * P, (su + 1) * P)
                        vps = l1ps.tile([P, 256], F32, name="v_ps", tag="v_ps")
                        for c in range(2):
                            nc.tensor.matmul(vps[:], r(xs[c][:, ssl]),
                                             r(wv_sb[c][:]),
                                             start=(c == 0), stop=False)
                        nc.tensor.matmul(vps[:], r(onesr[:]), r(bv_row[:]),
                                         start=False, stop=True)
                        nc.scalar.copy(vsb[su][:], vps[:])

                    # scoresT -> pT (exp) with running exp-sum partials
                    pt = [l1.tile([P, T], F32, name=f"pt{pr}", tag=f"pt{pr}") for pr in range(2)]
                    for pr in range(2):
                        scps = l1ps.tile([P, T], F32, name="sc_ps", tag="sc_ps", bufs=2)
                        nc.tensor.matmul(scps[:], r(k1blk[pr][:]), r(qt[pr][:]))
                        nc.scalar.activation(pt[pr][:], scps[:], AF.Exp,
                                             scale=SCALE,
                                             accum_out=scol[pr][:, ti:ti + 1])

                    # w1uT tiles (persist)
                    for pr in range(2):
                        wps = l1ps.tile([P, T], F32, name="w1u_ps", tag="w1u_ps")
                        nc.tensor.matmul(wps[:], r(a2blk[pr][:]), r(pt[pr][:]))
                        w1t = wp.tile([P, T], F32, name="w1u", tag="w1u")
                        nc.scalar.copy(w1t[:], wps[:])
                        w1uT[ti][pr] = w1t

                    # p_N via PE transpose; rowsums; w2numer accumulation
                    for su in range(NSUB):
                        ssl = slice(su * P, (su + 1) * P)
                        sug = ti * NSUB + su
                        pnps = l1ps.tile([P, 256], F32, name="pn_ps", tag="pn_ps")
                        for pr in range(2):
                            nc.tensor.matmul(pnps[:, pr * P:(pr + 1) * P],
                                             pt[pr][:, ssl], ident[:],
                                             is_transpose=True,
                                             skip_group_check=True)
                        pn = l1.tile([P, 256], F32, name="pn", tag="pn")
                        for h4 in range(4):
                            nc.scalar.activation(
                                pn[:, h4 * KK:(h4 + 1) * KK],
                                pnps[:, h4 * KK:(h4 + 1) * KK], AF.Identity,
                                accum_out=rs_N[:, sug * 4 + h4:sug * 4 + h4 + 1])
                        first = (sug == 0)
                        last = (sug == NTILES * NSUB - 1)
                        for pr in range(2):
                            nc.tensor.matmul(w2n_ps[pr][:],
                                             r(vsb[su][:, pr * P:(pr + 1) * P]),
                                             r(pn[:]),
                                             start=first, stop=last,
                                             skip_group_check=True)

                # drain partials to DRAM + collective
                for pr in range(2):
                    w2nsb = l1.tile([P, 256], F32, name=f"w2nsb{pr}", tag=f"w2nsb{pr}")
                    nc.vector.tensor_copy(w2nsb[:], w2n_ps[pr][:])
                    nc.sync.dma_start(partial[pr * P:(pr + 1) * P, :], w2nsb[:])
                    ssum = l1.tile([P, 1], F32, name=f"ssum{pr}", tag=f"ssum{pr}")
                    nc.vector.tensor_reduce(ssum[:], scol[pr][:], AX.X, ALU.add)
                    nc.sync.dma_start(
                        partial[256:257, pr * P:(pr + 1) * P], ssum[:])

            nc.gpsimd.collective_compute(
                "AllReduce", ALU.add,
                replica_groups=[[0, 1], [2, 3], [4, 5], [6, 7]],
                ins=[partial[:].opt()], outs=[reduced[:].opt()])

            # ---------------- w2blk + rinv prep ----------------
            red = [pp.tile([P, 256], F32, name=f"red{pr}", tag=f"red{pr}") for pr in range(2)]
            # sinv per (pair, head-block), each at partition base 0
            sinv = [[pp.tile([KK, 1], F32, name=f"sinv{pr}{hb}",
                             tag=f"sinv{pr}{hb}") for hb in range(2)]
                    for pr in range(2)]
            for pr in range(2):
                nc.sync.dma_start(red[pr][:], reduced[pr * P:(pr + 1) * P, :])
                for hb in range(2):
                    stmp = pp.tile([KK, 1], F32, name=f"stmp{pr}{hb}",
                                   tag=f"stmp{pr}{hb}")
                    off = pr * P + hb * KK
                    nc.sync.dma_start(stmp[:], reduced[256:257, off:off + KK])
                    nc.vector.reciprocal(sinv[pr][hb][:], stmp[:])

            w2blk = [pp.tile([P, P], F32, name=f"w2blk{pr}", tag=f"w2blk{pr}") for pr in range(2)]
            psB_cm = tc.tile_pool(name="psB", bufs=2, space="PSUM")
            psB = psB_cm.__enter__()
            for pr in range(2):
                nc.vector.memset(w2blk[pr][:], 0.0)
                for hb in range(2):
                    rsl = slice(hb * KK, (hb + 1) * KK)
                    csl = slice(pr * P + hb * KK, pr * P + (hb + 1) * KK)
                    tps = psB.tile([KK, KK], F32, name="w2t_ps",
                                   tag="w2t_ps", bufs=2)
                    nc.tensor.matmul(tps[:], red[pr][rsl, csl],
                                     ident[rsl, rsl], is_transpose=True)
                    stg = pp.tile([KK, KK], F32, name=f"w2stg{pr}{hb}",
                                  tag=f"w2stg{pr}{hb}")
                    nc.vector.tensor_scalar(stg[:], tps[:],
                                            sinv[pr][hb][:], None, ALU.mult)
                    nc.sync.dma_start(w2blk[pr][rsl, rsl], stg[:])

            rinv_N = pp.tile([P, 256], F32, name="rinvN", tag="rinvN")
            nc.vector.reciprocal(rinv_N[:], rs_N[:])
            rinvT_sb = [pp.tile([P, P], F32, name=f"rinvT{c}", tag=f"rinvT{c}")
                        for c in range(2)]
            for c in range(2):
                rtp = psB.tile([P, P], F32, name="rt_ps", tag="rt_ps", bufs=2)
                nc.tensor.matmul(rtp[:], rinv_N[:, c * P:(c + 1) * P],
                                 ident[:], is_transpose=True)
                nc.scalar.copy(rinvT_sb[c][:], rtp[:])
            psB_cm.__exit__(None, None, None)

            psB_cm.__exit__(None, None, None)

            # ---------------- loop 2 ----------------
            with tc.tile_pool(name="l2", bufs=2) as l2, \
                 tc.tile_pool(name="l2h", bufs=2) as l2h, \
                 tc.tile_pool(name="l2ps", bufs=1, space="PSUM") as l2ps, \
                 tc.tile_pool(name="stps", bufs=1, space="PSUM") as stps:

                def layernorm_rows(tag, ti, chunks, g_sb, be_sb, y_out):
                    """chunks: two (128,T) sbuf tiles (input). Writes
                    normalized result to y_out[2] (128,T) tiles."""
                    st = stps.tile([1, T], F32, name="st_ps", tag="st_ps")
                    stq = stps.tile([1, T], F32, name="stq_ps", tag="stq_ps")
                    for c in range(2):
                        nc.tensor.matmul(st[0:1, :], r(mcol[:]), r(chunks[c][:]),
                                         start=(c == 0), stop=(c == 1))
                    for c in range(2):
                        sq = l2.tile([P, T], F32, name="sq", tag="sq")
                        nc.scalar.square(sq[:], chunks[c][:])
                        nc.tensor.matmul(stq[0:1, :], r(mcol[:]), r(sq[:]),
                                         start=(c == 0), stop=(c == 1))
                    stsb = l2.tile([1, 2 * T], F32, name="stsb", tag="stsb")
                    nc.scalar.copy(stsb[0:1, 0:T], st[0:1, :])
                    nc.scalar.copy(stsb[0:1, T:2 * T], stq[0:1, :])
                    sd1 = dp.tile([2, T], F32, name="sd1", tag="sd1")
                    nc.sync.dma_start(sd1[:], stsb[:])
                    sf = l2.tile([P, 8], F32, name="sf", tag="sf")
                    nc.sync.dma_start(
                        sf[:], sd1[:].rearrange("two (p f) -> p two f", f=4))
                    m2t = l2.tile([P, 4], F32, name="m2t", tag="m2t")
                    nc.vector.tensor_tensor(m2t[:], sf[:, 0:4], sf[:, 0:4],
                                            ALU.mult)
                    var = l2.tile([P, 4], F32, name="var", tag="var")
                    nc.vector.tensor_tensor(var[:], sf[:, 4:8], m2t[:],
                                            ALU.subtract)
                    sdv = l2.tile([P, 4], F32, name="sdv", tag="sdv")
                    nc.scalar.activation(sdv[:], var[:], AF.Sqrt, bias=epsc[:, 0:1])
                    nc.vector.reciprocal(sf[:, 4:8], sdv[:])
                    sd2 = dp.tile([2, T], F32, name="sd2", tag="sd2")
                    nc.sync.dma_start(
                        sd2[:].rearrange("two (p f) -> p two f", f=4), sf[:])
                    mr = l2.tile([1, 2 * T], F32, name="mr", tag="mr")
                    nc.sync.dma_start(mr[:], sd2[:])
                    mb = l2ps.tile([P, T], F32, name="mb_ps", tag="mb_ps")
                    nc.tensor.matmul(mb[:], r(onesr[:]), r(mr[0:1, 0:T]))
                    rb = l2ps.tile([P, T], F32, name="rb_ps", tag="rb_ps")
                    nc.tensor.matmul(rb[:], r(onesr[:]), r(mr[0:1, T:2 * T]))
                    for c in range(2):
                        t1 = l2.tile([P, T], F32, name="lnt1", tag="lnt1")
                        nc.vector.tensor_tensor(t1[:], chunks[c][:], mb[:],
                                                ALU.subtract)
                        nc.vector.tensor_tensor(t1[:], t1[:], rb[:], ALU.mult)
                        nc.vector.tensor_scalar(y_out[c][:], t1[:],
                                                g_sb[:, c:c + 1],
                                                be_sb[:, c:c + 1],
                                                ALU.mult, ALU.add)

                for ti in range(NTILES):
                    tsl = slice(ti * T, (ti + 1) * T)
                    Bt = [l2.tile([P, T], F32, name=f"B{pr}", tag=f"B{pr}") for pr in range(2)]
                    rrt = l2.tile([4, T], F32, name="rrt", tag="rrt")
                    rc = ti // 8
                    a0 = ti * 4 - 32 * rc
                    rT3 = rinvT_sb[rc][:].rearrange("(a b) t -> a b t", b=4)
                    for h4 in range(4):
                        nc.sync.dma_start(
                            rrt[h4:h4 + 1, :].rearrange(
                                "o (su tp) -> o su tp", tp=P),
                            rT3[a0:a0 + 4, h4, :])
                    for pr in range(2):
                        ops = l2ps.tile([P, T], F32, name="o_ps", tag="o_ps")
                        nc.tensor.matmul(ops[:], r(w2blk[pr][:]),
                                         r(w1uT[ti][pr][:]))
                        rbps = l2ps.tile([P, T], F32, name="rinvb_ps", tag="rinvb_ps")
                        nc.tensor.matmul(rbps[:],
                                         r(esel[:, pr * P:(pr + 1) * P]),
                                         r(rrt[:]))
                        rbsb = l2.tile([P, T], F32, name="rbsb", tag="rbsb")
                        nc.scalar.copy(rbsb[:], rbps[:])
                        xp = l2.tile([P, T], F32, name=f"xp{pr}", tag=f"xp{pr}")
                        for j in range(2):
                            nc.sync.dma_start(xp[j * KK:(j + 1) * KK, :],
                                              xsP_v[2 * pr + j, :, tsl])
                        nc.vector.tensor_tensor(Bt[pr][:], ops[:], rbsb[:],
                                                ALU.mult)
                        nc.vector.tensor_tensor(Bt[pr][:], Bt[pr][:], xp[:],
                                                ALU.add)

                    ffn = [l2.tile([P, T], F32, name=f"ffn{c}", tag=f"ffn{c}") for c in range(2)]
                    layernorm_rows("ln1", ti, Bt, g1_sb, be1_sb, ffn)

                    hT = [l2h.tile([P, T], F32, name=f"h{m}", tag=f"h{m}") for m in range(4)]
                    for m in range(4):
                        hps = l2ps.tile([P, T], F32, name="h_ps", tag="h_ps")
                        for c in range(2):
                            nc.tensor.matmul(hps[:],
                                             r(w1_sb[c][:, m * P:(m + 1) * P]),
                                             r(ffn[c][:]),
                                             start=(c == 0), stop=(c == 1))
                        nc.scalar.activation(hT[m][:], hps[:], AF.Gelu,
                                             bias=b1_sb[:, m:m + 1])

                    r2 = [l2.tile([P, T], F32, name=f"r2{c}", tag=f"r2{c}") for c in range(2)]
                    for c in range(2):
                        rps = l2ps.tile([P, T], F32, name="r2_ps", tag="r2_ps")
                        for m in range(4):
                            nc.tensor.matmul(rps[:],
                                             r(w2_sb[m][:, c * P:(c + 1) * P]),
                                             r(hT[m][:]),
                                             start=(m == 0), stop=(m == 3))
                        nc.vector.scalar_tensor_tensor(r2[c][:], rps[:],
                                                       b2_sb[:, c:c + 1],
                                                       ffn[c][:],
                                                       ALU.add, ALU.add)

                    yt = [l2.tile([P, T], F32, name=f"y{c}", tag=f"y{c}") for c in range(2)]
                    layernorm_rows("ln2", ti, r2, g2_sb, be2_sb, yt)
                    for c in range(2):
                        for j in range(2):
                            nc.sync.dma_start(yP_v[2 * c + j, :, tsl],
                                              yt[c][j * KK:(j + 1) * KK, :])

    nc.compile()
    return nc


_CACHED = {}


def kernel(x, C, G=None, Wq=None, bq=None, Wk=None, bk=None, Wv=None, bv=None,
           W1=None, b1=None, W2=None, b2=None,
           ln1_g=None, ln1_b=None, ln2_g=None, ln2_b=None, **_ignored):
    x = np.ascontiguousarray(np.asarray(x, np.float32))
    C = np.ascontiguousarray(np.asarray(C, np.float32))
    B, DIM, N = x.shape[0], x.shape[1], x.shape[2]
    assert (B, DIM, N) == (4, 256, 16384), (B, DIM, N)

    if "nc" not in _CACHED:
        _CACHED["nc"] = build_program()
    nc = _CACHED["nc"]

    f32 = lambda a: np.ascontiguousarray(np.asarray(a, np.float32))
    shared = dict(
        Wq=f32(Wq), bq=f32(bq), Wk=f32(Wk), bk=f32(bk), Wv=f32(Wv), bv=f32(bv),
        W1=f32(W1), b1=f32(b1), W2=f32(W2), b2=f32(b2),
        ln1_g=f32(ln1_g), ln1_b=f32(ln1_b), ln2_g=f32(ln2_g), ln2_b=f32(ln2_b),
        ident=np.eye(P, dtype=np.float32),
        onesr=np.ones((1, P), np.float32),
        mcol=np.full((P, 1), 1.0 / DIMV, np.float32),
        esel=np.repeat(np.eye(4, dtype=np.float32), KK, axis=1),
    )
    in_maps = []
    for c in range(8):
        b, half = c // 2, c % 2
        n0 = half * NT
        m = dict(shared)
        m["xsT"] = np.ascontiguousarray(x[b, :, n0:n0 + NT, 0])
        m["Cb"] = np.ascontiguousarray(C[b])
        in_maps.append(m)

    res = run_bass_kernel_spmd(nc, in_maps, list(range(8)), trace=TRACE,
                               **TRACE_KW)
    _CACHED["last_result"] = res

    out = np.empty((B, DIM, N, 1), np.float32)
    for c in range(8):
        b, half = c // 2, c % 2
        n0 = half * NT
        out[b, :, n0:n0 + NT, 0] = res.results[c]["y"]
    return out


if __name__ == "__main__":
    nc = build_program()
    print("program built ok")



# revision 6
# speedup vs baseline: 1.2121x; 1.2121x over previous
"""Trainium2 Bass kernel for nn_ContentAgree (dense transformer block).

Sharding: 8 cores = 4 batches x 2 sequence-halves. Each core processes
8192 tokens of one batch through the full block. The only cross-core
dependency is the attn3 softmax over the full sequence (N=16384) and
w2 = attn3 @ V: cores compute exp-sum partials + numerator partials and
a pairwise AllReduce combines the two halves of each batch.

All matmul operands are bf16 (PE runs 1 cycle/row vs 4 for fp32);
accumulation stays fp32 in PSUM, and softmax denominators / LayerNorm
statistics / the AllReduce stay fp32. Host converts x/C/weights to bf16.

Layout conventions per core (partition dim first):
  xsT   (256 ch, 8192 tok)  "T-layout" - directly matches x[b] in DRAM
  QT    (256, T)  rows h*64+hd
  V     (T, 256)  token-major ("N-layout"), cols h*64+hd
  scoresT/pT (128 = [k of head pair], T)
  p_N   (T, 256)  cols k of 4 heads
  w1uT  (128 = [k' of head pair], 8192)   persists in SBUF
  out/ffn tiles (ch, T) with channels PERMUTED to (h,hd)-major; the
  reference merges heads as channel = hd*4+h, handled by permuted reads
  of x/W1/W2/ln vectors and a permuted final store (pure AP tricks).
"""
import sys

for _p in ("/root/.axon_site/_ro/trn_rl_repo", "/opt/trn_rl_repo"):
    if _p not in sys.path:
        sys.path.append(_p)

import numpy as np
import ml_dtypes
import concourse.bass as bass
import concourse.bacc as bacc
import concourse.tile as tile
from concourse import mybir
from concourse.bass_utils import run_bass_kernel_spmd

dt = mybir.dt
F32 = dt.float32
BF = dt.bfloat16
AF = mybir.ActivationFunctionType
ALU = mybir.AluOpType
AX = mybir.AxisListType

P = 128
T = 512                 # token tile width (free dim)
NT = 8192               # tokens per core
NTILES = NT // T        # 16
NSUB = T // P           # 4
DIMV = 256
KK = 64
SCALE = 0.125           # 1/sqrt(64)
EPS = 1e-5

TRACE = False
TRACE_KW = {}


def build_program():
    nc = bacc.Bacc("TRN2", target_bir_lowering=False, debug=False, num_devices=8)

    # ---------------- DRAM I/O ----------------
    xsT_d = nc.dram_tensor("xsT", [DIMV, NT], BF, kind="ExternalInput")
    cb_d = nc.dram_tensor("Cb", [KK, DIMV], BF, kind="ExternalInput")
    wq_d = nc.dram_tensor("Wq", [DIMV, DIMV], BF, kind="ExternalInput")
    wk_d = nc.dram_tensor("Wk", [DIMV, DIMV], BF, kind="ExternalInput")
    wv_d = nc.dram_tensor("Wv", [DIMV, DIMV], BF, kind="ExternalInput")
    bq_d = nc.dram_tensor("bq", [DIMV], F32, kind="ExternalInput")
    bk_d = nc.dram_tensor("bk", [DIMV], F32, kind="ExternalInput")
    bv_d = nc.dram_tensor("bv", [DIMV], BF, kind="ExternalInput")
    w1_d = nc.dram_tensor("W1", [DIMV, 2 * DIMV], BF, kind="ExternalInput")
    b1_d = nc.dram_tensor("b1", [2 * DIMV], F32, kind="ExternalInput")
    w2_d = nc.dram_tensor("W2", [2 * DIMV, DIMV], BF, kind="ExternalInput")
    b2_d = nc.dram_tensor("b2", [DIMV], F32, kind="ExternalInput")
    g1_d = nc.dram_tensor("ln1_g", [DIMV], F32, kind="ExternalInput")
    be1_d = nc.dram_tensor("ln1_b", [DIMV], F32, kind="ExternalInput")
    g2_d = nc.dram_tensor("ln2_g", [DIMV], F32, kind="ExternalInput")
    be2_d = nc.dram_tensor("ln2_b", [DIMV], F32, kind="ExternalInput")
    ident_d = nc.dram_tensor("ident", [P, P], F32, kind="ExternalInput")
    identb_d = nc.dram_tensor("identb", [P, P], BF, kind="ExternalInput")
    onesr_d = nc.dram_tensor("onesr", [1, P], BF, kind="ExternalInput")
    mcol_d = nc.dram_tensor("mcol", [P, 1], BF, kind="ExternalInput")
    esel_d = nc.dram_tensor("esel", [4, DIMV], BF, kind="ExternalInput")
    y_d = nc.dram_tensor("y", [DIMV, NT], F32, kind="ExternalOutput")


    # permuted DRAM views ((hd,h)-major channel -> (h,hd)-major), exposed as
    # 3-D APs (h, hd, ...) since rearrange cannot regroup non-adjacent dims;
    # DMA flattens nested dims, so a [2,64,T] view fills a [128,T] tile.
    xsP_v = xsT_d[:].rearrange("(hd h) n -> h hd n", h=4)
    w1P_v = w1_d[:].rearrange("(hd h) f -> h hd f", h=4)
    w2P_v = w2_d[:].rearrange("f (hd h) -> f h hd", h=4)
    yP_v = y_d[:].rearrange("(hd h) n -> h hd n", h=4)

    with tile.TileContext(nc) as tc:
        cpool = tc.tile_pool(name="consts", bufs=1)
        ppool = tc.tile_pool(name="persist", bufs=1)
        w1upool = tc.tile_pool(name="w1u", bufs=2 * NTILES)
        dpool = tc.tile_pool(name="drambounce", bufs=6, space="DRAM")
        with cpool as cp, ppool as pp, w1upool as wp, dpool as dp:
            # ---------------- load constants ----------------
            wk_sb = [cp.tile([P, DIMV], BF, name=f"wk{c}", tag=f"wk{c}") for c in range(2)]
            wv_sb = [cp.tile([P, DIMV], BF, name=f"wv{c}", tag=f"wv{c}") for c in range(2)]
            wq_sb = [cp.tile([P, DIMV], BF, name=f"wq{c}", tag=f"wq{c}") for c in range(2)]
            w1_sb = [cp.tile([P, 2 * DIMV], BF, name=f"w1{c}", tag=f"w1{c}") for c in range(2)]
            w2_sb = [cp.tile([P, DIMV], BF, name=f"w2{c}", tag=f"w2{c}") for c in range(4)]
            for c in range(2):
                nc.sync.dma_start(wk_sb[c][:], wk_d[c * P:(c + 1) * P, :])
                nc.sync.dma_start(wv_sb[c][:], wv_d[c * P:(c + 1) * P, :])
                nc.sync.dma_start(wq_sb[c][:], wq_d[c * P:(c + 1) * P, :])
                for j in range(2):
                    nc.sync.dma_start(w1_sb[c][j * KK:(j + 1) * KK, :],
                                      w1P_v[2 * c + j, :, :])
            for c in range(4):
                for j in range(4):
                    nc.sync.dma_start(w2_sb[c][:, j * KK:(j + 1) * KK],
                                      w2P_v[c * P:(c + 1) * P, j, :])

            bq_sb = cp.tile([P, 2], F32, name="bq", tag="bq")
            bk_sb = cp.tile([P, 2], F32, name="bk", tag="bk")
            b1_sb = cp.tile([P, 4], F32, name="b1", tag="b1")
            b2_sb = cp.tile([P, 2], F32, name="b2", tag="b2")
            g1_sb = cp.tile([P, 2], F32, name="g1", tag="g1")
            be1_sb = cp.tile([P, 2], F32, name="be1", tag="be1")
            g2_sb = cp.tile([P, 2], F32, name="g2", tag="g2")
            be2_sb = cp.tile([P, 2], F32, name="be2", tag="be2")
            nc.sync.dma_start(bq_sb[:], bq_d[:].rearrange("(c p) -> p c", p=P))
            nc.sync.dma_start(bk_sb[:], bk_d[:].rearrange("(c p) -> p c", p=P))
            nc.sync.dma_start(b1_sb[:], b1_d[:].rearrange("(m p) -> p m", p=P))
            for vd, vt in ((b2_d, b2_sb), (g1_d, g1_sb), (be1_d, be1_sb),
                           (g2_d, g2_sb), (be2_d, be2_sb)):
                vperm = vd[:].rearrange("(hd h) -> h hd", h=4)
                for c in range(2):
                    for j in range(2):
                        nc.sync.dma_start(vt[j * KK:(j + 1) * KK, c:c + 1],
                                          vperm[2 * c + j, :])

            bv_row = cp.tile([1, DIMV], BF, name="bvrow", tag="bvrow")
            nc.sync.dma_start(bv_row[:], bv_d[:].rearrange("(o d) -> o d", o=1))
            epsc = cp.tile([P, 1], F32, name="epsc", tag="epsc")
            nc.vector.memset(epsc[:], EPS)
            ident = cp.tile([P, P], F32, name="ident", tag="ident")
            identb = cp.tile([P, P], BF, name="identb", tag="identb")
            onesr = cp.tile([1, P], BF, name="onesr", tag="onesr")
            mcol = cp.tile([P, 1], BF, name="mcol", tag="mcol")
            esel = cp.tile([4, DIMV], BF, name="esel", tag="esel")
            nc.sync.dma_start(ident[:], ident_d[:])
            nc.sync.dma_start(identb[:], identb_d[:])
            nc.sync.dma_start(onesr[:], onesr_d[:])
            nc.sync.dma_start(mcol[:], mcol_d[:])
            nc.sync.dma_start(esel[:], esel_d[:])

            # ---------------- K1T / attn2 prep ----------------
            psA_cm = tc.tile_pool(name="psA", bufs=1, space="PSUM")
            psA = psA_cm.__enter__()
            psp = psA
            cb_sb = pp.tile([KK, DIMV], BF, name="cb", tag="cb")
            nc.sync.dma_start(cb_sb[:], cb_d[:])
            cbT0 = pp.tile([P, KK], BF, name="cbT0", tag="cbT0")
            cbT1 = pp.tile([P, KK], BF, name="cbT1", tag="cbT1")
            for c, cbTt in enumerate((cbT0, cbT1)):
                tp = psp.tile([P, KK], BF, name="cbT_ps", tag="cbT_ps")
                nc.tensor.matmul(tp[:], cb_sb[:, c * P:(c + 1) * P],
                                 identb[0:KK, 0:KK], is_transpose=True)
                nc.scalar.copy(cbTt[:], tp[:])

            k1t = [pp.tile([P, KK], BF, name=f"k1t{m}", tag=f"k1t{m}") for m in range(2)]
            for m in range(2):
                k1ps = psp.tile([P, KK], F32, name="k1_ps", tag="k1_ps")
                for c, cbTt in enumerate((cbT0, cbT1)):
                    nc.tensor.matmul(k1ps[:], wq_sb[c][:, m * P:(m + 1) * P],
                                     cbTt[:], start=(c == 0), stop=(c == 1))
                nc.scalar.activation(k1t[m][:], k1ps[:], AF.Identity,
                                     bias=bq_sb[:, m:m + 1])

            k1blk = [pp.tile([P, P], BF, name=f"k1blk{pr}", tag=f"k1blk{pr}") for pr in range(2)]
            for pr in range(2):
                nc.vector.memset(k1blk[pr][:], 0.0)
                nc.scalar.copy(k1blk[pr][0:KK, 0:KK], k1t[pr][0:KK, :])
                nc.scalar.copy(k1blk[pr][KK:P, KK:P], k1t[pr][KK:P, :])

            a2blk = [pp.tile([P, P], BF, name=f"a2blk{pr}", tag=f"a2blk{pr}") for pr in range(2)]
            for pr in range(2):
                scps = psp.tile([P, P], F32, name="a2_ps", tag="a2_ps")
                nc.tensor.matmul(scps[:], k1blk[pr][:], k1blk[pr][:])
                nc.vector.memset(a2blk[pr][:], 0.0)
                for hb in range(2):
                    sl = slice(hb * KK, (hb + 1) * KK)
                    mx = pp.tile([P, 1], F32, name="a2mx", tag="a2mx")
                    nc.vector.tensor_reduce(mx[sl, :], scps[sl, sl], AX.X, ALU.max)
                    nmx = pp.tile([P, 1], F32, name="a2nmx", tag="a2nmx")
                    nc.vector.tensor_scalar_mul(nmx[sl, :], mx[sl, :], -SCALE)
                    rsum = pp.tile([P, 1], F32, name="a2rs", tag="a2rs")
                    nc.scalar.activation(a2blk[pr][sl, sl], scps[sl, sl], AF.Exp,
                                         bias=nmx[sl, :], scale=SCALE,
                                         accum_out=rsum[sl, :])
                    rinv = pp.tile([P, 1], F32, name="a2ri", tag="a2ri")
                    nc.vector.reciprocal(rinv[sl, :], rsum[sl, :])
                    nc.vector.tensor_scalar(a2blk[pr][sl, sl], a2blk[pr][sl, sl],
                                            rinv[sl, :], None, ALU.mult)

            # persistent accumulators
            rs_N = pp.tile([P, 256], F32, name="rsN", tag="rsN")
            scol = [pp.tile([P, NTILES], F32, name=f"scol{pr}", tag=f"scol{pr}") for pr in range(2)]
            w1uT = [[None] * 2 for _ in range(NTILES)]

            # DRAM bounce tiles for the collective
            partial = dp.tile([257, 256], F32, name="partial", tag="partial")
            reduced = dp.tile([257, 256], F32, name="reduced", tag="reduced")

            psA_cm.__exit__(None, None, None)

            # ---------------- loop 1 ----------------
            with tc.tile_pool(name="l1", bufs=3) as l1, \
                 tc.tile_pool(name="l1ps", bufs=1, space="PSUM") as l1ps, \
                 tc.tile_pool(name="w2nps", bufs=1, space="PSUM") as w2nps:
                w2n_ps = [w2nps.tile([P, 256], F32, name=f"w2n{pr}", tag=f"w2n{pr}")
                          for pr in range(2)]
                for ti in range(NTILES):
                    tsl = slice(ti * T, (ti + 1) * T)
                    xs = [l1.tile([P, T], BF, name=f"xs{c}", tag=f"xs{c}") for c in range(2)]
                    for c in range(2):
                        nc.sync.dma_start(xs[c][:], xsT_d[c * P:(c + 1) * P, tsl])

                    qt = [l1.tile([P, T], BF, name=f"qt{m}", tag=f"qt{m}") for m in range(2)]
                    for m in range(2):
                        qtps = l1ps.tile([P, T], F32, name="qt_ps", tag="qt_ps")
                        for c in range(2):
                            nc.tensor.matmul(qtps[:],
                                             wk_sb[c][:, m * P:(m + 1) * P],
                                             xs[c][:],
                                             start=(c == 0), stop=(c == 1))
                        nc.scalar.activation(qt[m][:], qtps[:], AF.Identity,
                                             bias=bk_sb[:, m:m + 1])

                    # V (token-major) per 128-token subtile
                    vsb = [l1.tile([P, 256], BF, name=f"v{su}", tag=f"v{su}")
                           for su in range(NSUB)]
                    for su in range(NSUB):
                        ssl = slice(su * P, (su + 1) * P)
                        vps = l1ps.tile([P, 256], F32, name="v_ps", tag="v_ps")
                        for c in range(2):
                            nc.tensor.matmul(vps[:], xs[c][:, ssl],
                                             wv_sb[c][:],
                                             start=(c == 0), stop=False)
                        nc.tensor.matmul(vps[:], onesr[:], bv_row[:],
                                         start=False, stop=True)
                        nc.scalar.copy(vsb[su][:], vps[:])

                    # scoresT -> pT (exp) with running exp-sum partials
                    pt = [l1.tile([P, T], BF, name=f"pt{pr}", tag=f"pt{pr}") for pr in range(2)]
                    for pr in range(2):
                        scps = l1ps.tile([P, T], F32, name="sc_ps", tag="sc_ps", bufs=2)
                        nc.tensor.matmul(scps[:], k1blk[pr][:], qt[pr][:])
                        nc.scalar.activation(pt[pr][:], scps[:], AF.Exp,
                                             scale=SCALE,
                                             accum_out=scol[pr][:, ti:ti + 1])

                    # w1uT tiles (persist)
                    for pr in range(2):
                        wps = l1ps.tile([P, T], F32, name="w1u_ps", tag="w1u_ps")
                        nc.tensor.matmul(wps[:], a2blk[pr][:], pt[pr][:])
                        w1t = wp.tile([P, T], BF, name="w1u", tag="w1u")
                        nc.scalar.copy(w1t[:], wps[:])
                        w1uT[ti][pr] = w1t

                    # p_N via PE transpose; rowsums; w2numer accumulation
                    for su in range(NSUB):
                        ssl = slice(su * P, (su + 1) * P)
                        sug = ti * NSUB + su
                        pnps = l1ps.tile([P, 256], BF, name="pn_ps", tag="pn_ps")
                        for pr in range(2):
                            nc.tensor.matmul(pnps[:, pr * P:(pr + 1) * P],
                                             pt[pr][:, ssl], identb[:],
                                             is_transpose=True,
                                             skip_group_check=True)
                        pn = l1.tile([P, 256], BF, name="pn", tag="pn")
                        for h4 in range(4):
                            nc.scalar.activation(
                                pn[:, h4 * KK:(h4 + 1) * KK],
                                pnps[:, h4 * KK:(h4 + 1) * KK], AF.Identity,
                                accum_out=rs_N[:, sug * 4 + h4:sug * 4 + h4 + 1])
                        first = (sug == 0)
                        last = (sug == NTILES * NSUB - 1)
                        for pr in range(2):
                            nc.tensor.matmul(w2n_ps[pr][:],
                                             vsb[su][:, pr * P:(pr + 1) * P],
                                             pn[:],
                                             start=first, stop=last,
                                             skip_group_check=True)

                # drain partials to DRAM + collective
                for pr in range(2):
                    w2nsb = l1.tile([P, 256], F32, name=f"w2nsb{pr}", tag=f"w2nsb{pr}")
                    nc.vector.tensor_copy(w2nsb[:], w2n_ps[pr][:])
                    nc.sync.dma_start(partial[pr * P:(pr + 1) * P, :], w2nsb[:])
                    ssum = l1.tile([P, 1], F32, name=f"ssum{pr}", tag=f"ssum{pr}")
                    nc.vector.tensor_reduce(ssum[:], scol[pr][:], AX.X, ALU.add)
                    nc.sync.dma_start(
                        partial[256:257, pr * P:(pr + 1) * P], ssum[:])

            nc.gpsimd.collective_compute(
                "AllReduce", ALU.add,
                replica_groups=[[0, 1], [2, 3], [4, 5], [6, 7]],
                ins=[partial[:].opt()], outs=[reduced[:].opt()])

            # ---------------- w2blk + rinv prep ----------------
            red = [pp.tile([P, 256], F32, name=f"red{pr}", tag=f"red{pr}") for pr in range(2)]
            # sinv per (pair, head-block), each at partition base 0
            sinv = [[pp.tile([KK, 1], F32, name=f"sinv{pr}{hb}",
                             tag=f"sinv{pr}{hb}") for hb in range(2)]
                    for pr in range(2)]
            for pr in range(2):
                nc.sync.dma_start(red[pr][:], reduced[pr * P:(pr + 1) * P, :])
                for hb in range(2):
                    stmp = pp.tile([KK, 1], F32, name=f"stmp{pr}{hb}",
                                   tag=f"stmp{pr}{hb}")
                    off = pr * P + hb * KK
                    nc.sync.dma_start(stmp[:], reduced[256:257, off:off + KK])
                    nc.vector.reciprocal(sinv[pr][hb][:], stmp[:])

            w2blk = [pp.tile([P, P], BF, name=f"w2blk{pr}", tag=f"w2blk{pr}") for pr in range(2)]
            psB_cm = tc.tile_pool(name="psB", bufs=2, space="PSUM")
            psB = psB_cm.__enter__()
            for pr in range(2):
                nc.vector.memset(w2blk[pr][:], 0.0)
                for hb in range(2):
                    rsl = slice(hb * KK, (hb + 1) * KK)
                    csl = slice(pr * P + hb * KK, pr * P + (hb + 1) * KK)
                    tps = psB.tile([KK, KK], F32, name="w2t_ps",
                                   tag="w2t_ps", bufs=2)
                    nc.tensor.matmul(tps[:], red[pr][rsl, csl],
                                     ident[rsl, rsl], is_transpose=True)
                    stg = pp.tile([KK, KK], BF, name=f"w2stg{pr}{hb}",
                                  tag=f"w2stg{pr}{hb}")
                    nc.vector.tensor_scalar(stg[:], tps[:],
                                            sinv[pr][hb][:], None, ALU.mult)
                    nc.sync.dma_start(w2blk[pr][rsl, rsl], stg[:])

            rinv_N = pp.tile([P, 256], F32, name="rinvN", tag="rinvN")
            nc.vector.reciprocal(rinv_N[:], rs_N[:])
            rinvT_sb = [pp.tile([P, P], BF, name=f"rinvT{c}", tag=f"rinvT{c}")
                        for c in range(2)]
            for c in range(2):
                rtp = psB.tile([P, P], F32, name="rt_ps", tag="rt_ps", bufs=2)
                nc.tensor.matmul(rtp[:], rinv_N[:, c * P:(c + 1) * P],
                                 ident[:], is_transpose=True)
                nc.scalar.copy(rinvT_sb[c][:], rtp[:])
            psB_cm.__exit__(None, None, None)

            # ---------------- loop 2 ----------------
            with tc.tile_pool(name="l2", bufs=2) as l2, \
                 tc.tile_pool(name="l2h", bufs=2) as l2h, \
                 tc.tile_pool(name="l2ps", bufs=1, space="PSUM") as l2ps, \
                 tc.tile_pool(name="stps", bufs=1, space="PSUM") as stps:

                def layernorm_rows(tag, ti, chunks, g_sb, be_sb, y_out, ydt):
                    """chunks: two (128,T) bf16 sbuf tiles (input). Writes
                    normalized result to y_out[2] (128,T) tiles of dtype ydt."""
                    st = stps.tile([1, T], F32, name="st_ps", tag="st_ps")
                    stq = stps.tile([1, T], F32, name="stq_ps", tag="stq_ps")
                    for c in range(2):
                        nc.tensor.matmul(st[0:1, :], mcol[:], chunks[c][:],
                                         start=(c == 0), stop=(c == 1))
                    for c in range(2):
                        sq = l2.tile([P, T], BF, name="sq", tag="sq")
                        nc.scalar.square(sq[:], chunks[c][:])
                        nc.tensor.matmul(stq[0:1, :], mcol[:], sq[:],
                                         start=(c == 0), stop=(c == 1))
                    stsb = l2.tile([1, 2 * T], F32, name="stsb", tag="stsb")
                    nc.scalar.copy(stsb[0:1, 0:T], st[0:1, :])
                    nc.scalar.copy(stsb[0:1, T:2 * T], stq[0:1, :])
                    sd1 = dp.tile([2, T], F32, name="sd1", tag="sd1")
                    nc.sync.dma_start(sd1[:], stsb[:])
                    sf = l2.tile([P, 8], F32, name="sf", tag="sf")
                    nc.sync.dma_start(
                        sf[:], sd1[:].rearrange("two (p f) -> p two f", f=4))
                    m2t = l2.tile([P, 4], F32, name="m2t", tag="m2t")
                    nc.vector.tensor_tensor(m2t[:], sf[:, 0:4], sf[:, 0:4],
                                            ALU.mult)
                    var = l2.tile([P, 4], F32, name="var", tag="var")
                    nc.vector.tensor_tensor(var[:], sf[:, 4:8], m2t[:],
                                            ALU.subtract)
                    sdv = l2.tile([P, 4], F32, name="sdv", tag="sdv")
                    nc.scalar.activation(sdv[:], var[:], AF.Sqrt, bias=epsc[:, 0:1])
                    nc.vector.reciprocal(sf[:, 4:8], sdv[:])
                    sfb = l2.tile([P, 8], BF, name="sfb", tag="sfb")
                    nc.vector.tensor_copy(sfb[:], sf[:])
                    sd2 = dp.tile([2, T], BF, name="sd2", tag="sd2")
                    nc.sync.dma_start(
                        sd2[:].rearrange("two (p f) -> p two f", f=4), sfb[:])
                    mr = l2.tile([1, 2 * T], BF, name="mr", tag="mr")
                    nc.sync.dma_start(mr[:], sd2[:])
                    mb = l2ps.tile([P, T], F32, name="mb_ps", tag="mb_ps")
                    nc.tensor.matmul(mb[:], onesr[:], mr[0:1, 0:T])
                    rb = l2ps.tile([P, T], F32, name="rb_ps", tag="rb_ps")
                    nc.tensor.matmul(rb[:], onesr[:], mr[0:1, T:2 * T])
                    for c in range(2):
                        t1 = l2.tile([P, T], BF, name="lnt1", tag="lnt1")
                        nc.vector.tensor_tensor(t1[:], chunks[c][:], mb[:],
                                                ALU.subtract)
                        nc.vector.tensor_tensor(t1[:], t1[:], rb[:], ALU.mult)
                        nc.vector.tensor_scalar(y_out[c][:], t1[:],
                                                g_sb[:, c:c + 1],
                                                be_sb[:, c:c + 1],
                                                ALU.mult, ALU.add)

                for ti in range(NTILES):
                    tsl = slice(ti * T, (ti + 1) * T)
                    Bt = [l2.tile([P, T], BF, name=f"B{pr}", tag=f"B{pr}") for pr in range(2)]
                    rrt = l2.tile([4, T], BF, name="rrt", tag="rrt")
                    rc = ti // 8
                    a0 = ti * 4 - 32 * rc
                    rT3 = rinvT_sb[rc][:].rearrange("(a b) t -> a b t", b=4)
                    for h4 in range(4):
                        nc.sync.dma_start(
                            rrt[h4:h4 + 1, :].rearrange(
                                "o (su tp) -> o su tp", tp=P),
                            rT3[a0:a0 + 4, h4, :])
                    for pr in range(2):
                        ops = l2ps.tile([P, T], F32, name="o_ps", tag="o_ps")
                        nc.tensor.matmul(ops[:], w2blk[pr][:],
                                         w1uT[ti][pr][:])
                        rbps = l2ps.tile([P, T], F32, name="rinvb_ps", tag="rinvb_ps")
                        nc.tensor.matmul(rbps[:],
                                         esel[:, pr * P:(pr + 1) * P],
                                         rrt[:])
                        rbsb = l2.tile([P, T], F32, name="rbsb", tag="rbsb")
                        nc.scalar.copy(rbsb[:], rbps[:])
                        xp = l2.tile([P, T], BF, name=f"xp{pr}", tag=f"xp{pr}")
                        for j in range(2):
                            nc.sync.dma_start(xp[j * KK:(j + 1) * KK, :],
                                              xsP_v[2 * pr + j, :, tsl])
                        nc.vector.tensor_tensor(Bt[pr][:], ops[:], rbsb[:],
                                                ALU.mult)
                        nc.vector.tensor_tensor(Bt[pr][:], Bt[pr][:], xp[:],
                                                ALU.add)

                    ffn = [l2.tile([P, T], BF, name=f"ffn{c}", tag=f"ffn{c}") for c in range(2)]
                    layernorm_rows("ln1", ti, Bt, g1_sb, be1_sb, ffn, BF)

                    hT = [l2h.tile([P, T], BF, name=f"h{m}", tag=f"h{m}") for m in range(4)]
                    for m in range(4):
                        hps = l2ps.tile([P, T], F32, name="h_ps", tag="h_ps")
                        for c in range(2):
                            nc.tensor.matmul(hps[:],
                                             w1_sb[c][:, m * P:(m + 1) * P],
                                             ffn[c][:],
                                             start=(c == 0), stop=(c == 1))
                        nc.scalar.activation(hT[m][:], hps[:], AF.Gelu,
                                             bias=b1_sb[:, m:m + 1])

                    r2 = [l2.tile([P, T], BF, name=f"r2{c}", tag=f"r2{c}") for c in range(2)]
                    for c in range(2):
                        rps = l2ps.tile([P, T], F32, name="r2_ps", tag="r2_ps")
                        for m in range(4):
                            nc.tensor.matmul(rps[:],
                                             w2_sb[m][:, c * P:(c + 1) * P],
                                             hT[m][:],
                                             start=(m == 0), stop=(m == 3))
                        nc.vector.scalar_tensor_tensor(r2[c][:], rps[:],
                                                       b2_sb[:, c:c + 1],
                                                       ffn[c][:],
                                                       ALU.add, ALU.add)

                    yt = [l2.tile([P, T], F32, name=f"y{c}", tag=f"y{c}") for c in range(2)]
                    layernorm_rows("ln2", ti, r2, g2_sb, be2_sb, yt, F32)
                    for c in range(2):
                        for j in range(2):
                            nc.sync.dma_start(yP_v[2 * c + j, :, tsl],
                                              yt[c][j * KK:(j + 1) * KK, :])

    nc.compile()
    return nc


_CACHED = {}


def kernel(x, C, G=None, Wq=None, bq=None, Wk=None, bk=None, Wv=None, bv=None,
           W1=None, b1=None, W2=None, b2=None,
           ln1_g=None, ln1_b=None, ln2_g=None, ln2_b=None, **_ignored):
    x = np.ascontiguousarray(np.asarray(x, np.float32))
    C = np.asarray(C, np.float32)
    B, DIM, N = x.shape[0], x.shape[1], x.shape[2]
    assert (B, DIM, N) == (4, 256, 16384), (B, DIM, N)

    if "nc" not in _CACHED:
        _CACHED["nc"] = build_program()
    nc = _CACHED["nc"]

    bf = ml_dtypes.bfloat16
    f32 = lambda a: np.ascontiguousarray(np.asarray(a, np.float32))
    b16 = lambda a: np.ascontiguousarray(np.asarray(a, np.float32).astype(bf))
    shared = dict(
        Wq=b16(Wq), bq=f32(bq), Wk=b16(Wk), bk=f32(bk), Wv=b16(Wv), bv=b16(bv),
        W1=b16(W1), b1=f32(b1), W2=b16(W2), b2=f32(b2),
        ln1_g=f32(ln1_g), ln1_b=f32(ln1_b), ln2_g=f32(ln2_g), ln2_b=f32(ln2_b),
        ident=np.eye(P, dtype=np.float32),
        identb=np.eye(P, dtype=np.float32).astype(bf),
        onesr=np.ones((1, P), np.float32).astype(bf),
        mcol=np.full((P, 1), 1.0 / DIMV, np.float32).astype(bf),
        esel=np.repeat(np.eye(4, dtype=np.float32), KK, axis=1).astype(bf),
    )
    xb = x[..., 0].astype(bf)  # (B, DIM, N) bf16
    in_maps = []
    for c in range(8):
        b, half = c // 2, c % 2
        n0 = half * NT
        m = dict(shared)
        m["xsT"] = np.ascontiguousarray(xb[b, :, n0:n0 + NT])
        m["Cb"] = b16(C[b])
        in_maps.append(m)

    res = run_bass_kernel_spmd(nc, in_maps, list(range(8)), trace=TRACE,
                               **TRACE_KW)
    _CACHED["last_result"] = res

    out = np.empty((B, DIM, N, 1), np.float32)
    for c in range(8):
        b, half = c // 2, c % 2
        n0 = half * NT
        out[b, :, n0:n0 + NT, 0] = res.results[c]["y"]
    return out


if __name__ == "__main__":
    nc = build_program()
    print("program built ok")


# revision 24
# speedup vs baseline: 2.0343x; 1.6783x over previous
"""Trainium2 Bass kernel for nn_ContentAgree (dense transformer block).

Sharding: 8 cores = 4 batches x 2 sequence-halves. Each core processes
8192 tokens of one batch through the full block. The only cross-core
dependency is the attn3 softmax over the full sequence (N=16384) and
w2 = attn3 @ V: cores compute exp-sum partials + numerator partials and
a pairwise AllReduce combines the two halves of each batch.

All matmul operands are bf16 (PE runs 1 cycle/row vs 4 for fp32);
accumulation stays fp32 in PSUM. Softmax denominators, the AllReduce
and the final LayerNorm (LN2) stay fp32 for precision margin.

Host pre-permutes weights / x / y layouts so every DMA is
row-contiguous (strided gathers ran ~20x slower than HBM speed).

attn1 normalization (1/rowsum over k) is computed in loop 1 via a tiny
mask-matmul on the PE and folded into the persistent w1uT tiles, so
loop 2 needs no per-token rescale machinery.

Loop 2 runs as three passes (A: attention-out + LN1, B: FFN + gelu,
C: LN2 + store) so the Scalar engine's activation-table switches drop
from 2/tile to ~4 total; LayerNorm statistics are processed as [1,T]
rows on-chip (no DRAM bounce).
"""
import sys

for _p in ("/root/.axon_site/_ro/trn_rl_repo", "/opt/trn_rl_repo"):
    if _p not in sys.path:
        sys.path.append(_p)

import numpy as np
import ml_dtypes
import concourse.bass as bass
import concourse.bacc as bacc
import concourse.tile as tile
from concourse import mybir
from concourse.bass_utils import run_bass_kernel_spmd

dt = mybir.dt
F32 = dt.float32
BF = dt.bfloat16
AF = mybir.ActivationFunctionType
ALU = mybir.AluOpType
AX = mybir.AxisListType

P = 128
T = 512                 # token tile width (free dim)
NT = 8192               # tokens per core
NTILES = NT // T        # 16
NSUB = T // P           # 4
DIMV = 256
KK = 64
SCALE = 0.125           # 1/sqrt(64)
EPS = 1e-5

TRACE = False
TRACE_KW = {}


def build_program():
    nc = bacc.Bacc("TRN2", target_bir_lowering=False, debug=False, num_devices=8)

    # ---------------- DRAM I/O (all host-side pre-permuted/contiguous) ----
    xsT_d = nc.dram_tensor("xsT", [DIMV, NT], BF, kind="ExternalInput")
    xsP_d = nc.dram_tensor("xsP", [DIMV, NT], BF, kind="ExternalInput")
    cb_d = nc.dram_tensor("Cb", [KK, DIMV], BF, kind="ExternalInput")
    wq_d = nc.dram_tensor("Wq", [DIMV, DIMV], BF, kind="ExternalInput")
    wk_d = nc.dram_tensor("Wk", [DIMV, DIMV], BF, kind="ExternalInput")
    wv_d = nc.dram_tensor("Wv", [DIMV, DIMV], BF, kind="ExternalInput")
    bq_d = nc.dram_tensor("bq", [P, 2], F32, kind="ExternalInput")
    bk_d = nc.dram_tensor("bk", [P, 2], F32, kind="ExternalInput")
    bv_d = nc.dram_tensor("bv", [1, DIMV], BF, kind="ExternalInput")
    w1_d = nc.dram_tensor("W1", [DIMV, 2 * DIMV], BF, kind="ExternalInput")
    b1_d = nc.dram_tensor("b1", [P, 4], F32, kind="ExternalInput")
    w2_d = nc.dram_tensor("W2", [2 * DIMV, DIMV], BF, kind="ExternalInput")
    b2_d = nc.dram_tensor("b2", [P, 2], F32, kind="ExternalInput")
    g1_d = nc.dram_tensor("ln1_g", [P, 2], F32, kind="ExternalInput")
    be1_d = nc.dram_tensor("ln1_b", [P, 2], F32, kind="ExternalInput")
    g2_d = nc.dram_tensor("ln2_g", [P, 2], F32, kind="ExternalInput")
    be2_d = nc.dram_tensor("ln2_b", [P, 2], F32, kind="ExternalInput")
    ident_d = nc.dram_tensor("ident", [P, P], F32, kind="ExternalInput")
    identb_d = nc.dram_tensor("identb", [P, P], BF, kind="ExternalInput")
    onesr_d = nc.dram_tensor("onesr", [1, P], BF, kind="ExternalInput")
    onesrf_d = nc.dram_tensor("onesrf", [1, P], F32, kind="ExternalInput")
    mcol_d = nc.dram_tensor("mcol", [P, 1], BF, kind="ExternalInput")
    esel_d = nc.dram_tensor("esel", [4, DIMV], BF, kind="ExternalInput")
    hbsel_d = nc.dram_tensor("hbsel", [P, 8], BF, kind="ExternalInput")
    y_d = nc.dram_tensor("y", [DIMV, NT], F32, kind="ExternalOutput")

    lp_cm = nc.allow_low_precision(
        reason="bf16 matmul pipeline; fp32 kept in PSUM accum/softmax/LN2")
    lp_cm.__enter__()
    with tile.TileContext(nc) as tc:
        cpool = tc.tile_pool(name="consts", bufs=1)
        ppool = tc.tile_pool(name="persist", bufs=1)
        dpool = tc.tile_pool(name="drambounce", bufs=4, space="DRAM")
        with cpool as cp, ppool as pp, dpool as dp:
            # ---------------- load constants (contiguous DMAs) ----------
            wk_sb = [cp.tile([P, DIMV], BF, name=f"wk{c}", tag=f"wk{c}") for c in range(2)]
            wv_sb = [cp.tile([P, DIMV], BF, name=f"wv{c}", tag=f"wv{c}") for c in range(2)]
            wq_sb = [cp.tile([P, DIMV], BF, name=f"wq{c}", tag=f"wq{c}") for c in range(2)]
            w1_sb = [cp.tile([P, 2 * DIMV], BF, name=f"w1{c}", tag=f"w1{c}") for c in range(2)]
            w2_sb = [cp.tile([P, DIMV], BF, name=f"w2{c}", tag=f"w2{c}") for c in range(4)]
            for c in range(2):
                nc.sync.dma_start(wk_sb[c][:], wk_d[c * P:(c + 1) * P, :])
                nc.sync.dma_start(wv_sb[c][:], wv_d[c * P:(c + 1) * P, :])
                nc.sync.dma_start(wq_sb[c][:], wq_d[c * P:(c + 1) * P, :])
                nc.sync.dma_start(w1_sb[c][:], w1_d[c * P:(c + 1) * P, :])
            for c in range(4):
                nc.sync.dma_start(w2_sb[c][:], w2_d[c * P:(c + 1) * P, :])

            bq_sb = cp.tile([P, 2], F32, name="bq", tag="bq")
            bk_sb = cp.tile([P, 2], F32, name="bk", tag="bk")
            b1_sb = cp.tile([P, 4], F32, name="b1", tag="b1")
            b2_sb = cp.tile([P, 2], F32, name="b2", tag="b2")
            g1_sb = cp.tile([P, 2], F32, name="g1", tag="g1")
            be1_sb = cp.tile([P, 2], F32, name="be1", tag="be1")
            g2_sb = cp.tile([P, 2], F32, name="g2", tag="g2")
            be2_sb = cp.tile([P, 2], F32, name="be2", tag="be2")
            for vd, vt in ((bq_d, bq_sb), (bk_d, bk_sb), (b1_d, b1_sb),
                           (b2_d, b2_sb), (g1_d, g1_sb), (be1_d, be1_sb),
                           (g2_d, g2_sb), (be2_d, be2_sb)):
                nc.sync.dma_start(vt[:], vd[:])

            bv_row = cp.tile([1, DIMV], BF, name="bvrow", tag="bvrow")
            nc.sync.dma_start(bv_row[:], bv_d[:])
            epsc = cp.tile([P, 1], F32, name="epsc", tag="epsc")
            nc.vector.memset(epsc[:], EPS)
            ident = cp.tile([P, P], F32, name="ident", tag="ident")
            identb = cp.tile([P, P], BF, name="identb", tag="identb")
            onesr = cp.tile([1, P], BF, name="onesr", tag="onesr")
            onesrf = cp.tile([1, P], F32, name="onesrf", tag="onesrf")
            mcol = cp.tile([P, 1], BF, name="mcol", tag="mcol")
            esel = cp.tile([4, DIMV], BF, name="esel", tag="esel")
            hbsel = cp.tile([P, 8], BF, name="hbsel", tag="hbsel")
            nc.sync.dma_start(ident[:], ident_d[:])
            nc.sync.dma_start(identb[:], identb_d[:])
            nc.sync.dma_start(onesr[:], onesr_d[:])
            nc.sync.dma_start(onesrf[:], onesrf_d[:])
            nc.sync.dma_start(mcol[:], mcol_d[:])
            nc.sync.dma_start(esel[:], esel_d[:])
            nc.sync.dma_start(hbsel[:], hbsel_d[:])

            # ---------------- K1T / attn2 prep ----------------
            psA_cm = tc.tile_pool(name="psA", bufs=1, space="PSUM")
            psp = psA_cm.__enter__()
            cb_sb = pp.tile([KK, DIMV], BF, name="cb", tag="cb")
            nc.sync.dma_start(cb_sb[:], cb_d[:])
            cbT0 = pp.tile([P, KK], BF, name="cbT0", tag="cbT0")
            cbT1 = pp.tile([P, KK], BF, name="cbT1", tag="cbT1")
            for c, cbTt in enumerate((cbT0, cbT1)):
                tp = psp.tile([P, KK], BF, name="cbT_ps", tag="cbT_ps")
                nc.tensor.matmul(tp[:], cb_sb[:, c * P:(c + 1) * P],
                                 identb[0:KK, 0:KK], is_transpose=True)
                nc.scalar.copy(cbTt[:], tp[:])

            k1t = [pp.tile([P, KK], BF, name=f"k1t{m}", tag=f"k1t{m}") for m in range(2)]
            for m in range(2):
                k1ps = psp.tile([P, KK], F32, name="k1_ps", tag="k1_ps")
                for c, cbTt in enumerate((cbT0, cbT1)):
                    nc.tensor.matmul(k1ps[:], wq_sb[c][:, m * P:(m + 1) * P],
                                     cbTt[:], start=(c == 0), stop=(c == 1))
                nc.scalar.activation(k1t[m][:], k1ps[:], AF.Identity,
                                     bias=bq_sb[:, m:m + 1])

            k1blk = [pp.tile([P, P], BF, name=f"k1blk{pr}", tag=f"k1blk{pr}") for pr in range(2)]
            for pr in range(2):
                nc.vector.memset(k1blk[pr][:], 0.0)
                nc.scalar.copy(k1blk[pr][0:KK, 0:KK], k1t[pr][0:KK, :])
                nc.scalar.copy(k1blk[pr][KK:P, KK:P], k1t[pr][KK:P, :])

            a2blk = [pp.tile([P, P], BF, name=f"a2blk{pr}", tag=f"a2blk{pr}") for pr in range(2)]
            for pr in range(2):
                scps = psp.tile([P, P], F32, name="a2_ps", tag="a2_ps")
                nc.tensor.matmul(scps[:], k1blk[pr][:], k1blk[pr][:])
                nc.vector.memset(a2blk[pr][:], 0.0)
                for hb in range(2):
                    sl = slice(hb * KK, (hb + 1) * KK)
                    mx = pp.tile([P, 1], F32, name="a2mx", tag="a2mx")
                    nc.vector.tensor_reduce(mx[sl, :], scps[sl, sl], AX.X, ALU.max)
                    nmx = pp.tile([P, 1], F32, name="a2nmx", tag="a2nmx")
                    nc.vector.tensor_scalar_mul(nmx[sl, :], mx[sl, :], -SCALE)
                    rsum = pp.tile([P, 1], F32, name="a2rs", tag="a2rs")
                    nc.scalar.activation(a2blk[pr][sl, sl], scps[sl, sl], AF.Exp,
                                         bias=nmx[sl, :], scale=SCALE,
                                         accum_out=rsum[sl, :])
                    rinv = pp.tile([P, 1], F32, name="a2ri", tag="a2ri")
                    nc.vector.reciprocal(rinv[sl, :], rsum[sl, :])
                    nc.vector.tensor_scalar(a2blk[pr][sl, sl], a2blk[pr][sl, sl],
                                            rinv[sl, :], None, ALU.mult)

            # persistent accumulators
            scol = [pp.tile([P, NTILES], F32, name=f"scol{pr}", tag=f"scol{pr}") for pr in range(2)]
            w1uT = [[None] * 2 for _ in range(NTILES)]

            # DRAM bounce tiles for the collective
            partial = dp.tile([257, 256], F32, name="partial", tag="partial")
            reduced = dp.tile([257, 256], F32, name="reduced", tag="reduced")

            psA_cm.__exit__(None, None, None)

            # ---------------- loop 1 ----------------
            w1up_cm = tc.tile_pool(name="w1u", bufs=2 * NTILES)
            wp = w1up_cm.__enter__()
            with tc.tile_pool(name="l1", bufs=3) as l1, \
                 tc.tile_pool(name="l1ps", bufs=1, space="PSUM") as l1ps, \
                 tc.tile_pool(name="w2nps", bufs=1, space="PSUM") as w2nps:
                w2n_all = w2nps.tile([P, 512], F32, name="w2n", tag="w2n")
                w2n_ps = [w2n_all[:, pr * 256:(pr + 1) * 256] for pr in range(2)]
                for ti in range(NTILES):
                    tsl = slice(ti * T, (ti + 1) * T)
                    xs = [l1.tile([P, T], BF, name=f"xs{c}", tag=f"xs{c}") for c in range(2)]
                    for c in range(2):
                        nc.sync.dma_start(xs[c][:], xsT_d[c * P:(c + 1) * P, tsl])

                    qt = [l1.tile([P, T], BF, name=f"qt{m}", tag=f"qt{m}") for m in range(2)]
                    for m in range(2):
                        qtps = l1ps.tile([P, T], F32, name="qt_ps", tag="qt_ps")
                        for c in range(2):
                            nc.tensor.matmul(qtps[:],
                                             wk_sb[c][:, m * P:(m + 1) * P],
                                             xs[c][:],
                                             start=(c == 0), stop=(c == 1))
                        nc.vector.tensor_scalar(qt[m][:], qtps[:],
                                                bk_sb[:, m:m + 1], None, ALU.add)

                    # V (token-major) per 128-token subtile
                    vsb = [l1.tile([P, 256], BF, name=f"v{su}", tag=f"v{su}")
                           for su in range(NSUB)]
                    for su in range(NSUB):
                        ssl = slice(su * P, (su + 1) * P)
                        vps = l1ps.tile([P, 256], F32, name="v_ps", tag="v_ps")
                        for c in range(2):
                            nc.tensor.matmul(vps[:], xs[c][:, ssl],
                                             wv_sb[c][:],
                                             start=(c == 0), stop=False)
                        nc.tensor.matmul(vps[:], onesr[:], bv_row[:],
                                         start=False, stop=True)
                        nc.scalar.copy(vsb[su][:], vps[:])

                    # scoresT -> pT (exp) with running exp-sum partials
                    pt = [l1.tile([P, T], BF, name=f"pt{pr}", tag=f"pt{pr}") for pr in range(2)]
                    for pr in range(2):
                        scps = l1ps.tile([P, T], F32, name="sc_ps", tag="sc_ps")
                        nc.tensor.matmul(scps[:], k1blk[pr][:], qt[pr][:])
                        nc.scalar.activation(pt[pr][:], scps[:], AF.Exp,
                                             scale=SCALE,
                                             accum_out=scol[pr][:, ti:ti + 1])

                    # attn1 denominators: per-head rowsums over k via
                    # mask-matmul; rows (pr,hb) = global head index
                    rs4 = l1ps.tile([4, T], F32, name="rs4", tag="rs4")
                    for pr in range(2):
                        nc.tensor.matmul(rs4[:], hbsel[:, pr * 4:(pr + 1) * 4],
                                         pt[pr][:],
                                         start=(pr == 0), stop=(pr == 1))
                    rinv4 = l1.tile([4, T], BF, name="rinv4", tag="rinv4")
                    nc.vector.reciprocal(rinv4[:], rs4[:])

                    # w1uT tiles (persist), with attn1 normalization folded in
                    for pr in range(2):
                        rbps = l1ps.tile([P, T], F32, name="rb_l1", tag="rb_l1")
                        nc.tensor.matmul(rbps[:], esel[:, pr * P:(pr + 1) * P],
                                         rinv4[:])
                        rbsb = l1.tile([P, T], BF, name="rbsb", tag="rbsb")
                        nc.scalar.copy(rbsb[:], rbps[:])
                        wps = l1ps.tile([P, T], F32, name="w1u_ps", tag="w1u_ps")
                        nc.tensor.matmul(wps[:], a2blk[pr][:], pt[pr][:])
                        w1t = wp.tile([P, T], BF, name="w1u", tag="w1u")
                        nc.vector.tensor_tensor(w1t[:], wps[:], rbsb[:], ALU.mult)
                        w1uT[ti][pr] = w1t

                    # p_N via PE transpose; w2numer accumulation
                    for su in range(NSUB):
                        ssl = slice(su * P, (su + 1) * P)
                        sug = ti * NSUB + su
                        pnps = l1ps.tile([P, 256], BF, name="pn_ps", tag="pn_ps")
                        for pr in range(2):
                            nc.tensor.matmul(pnps[:, pr * P:(pr + 1) * P],
                                             pt[pr][:, ssl], identb[:],
                                             is_transpose=True,
                                             skip_group_check=True)
                        pn = l1.tile([P, 256], BF, name="pn", tag="pn")
                        nc.vector.tensor_copy(pn[:], pnps[:])
                        first = (sug == 0)
                        last = (sug == NTILES * NSUB - 1)
                        for pr in range(2):
                            nc.tensor.matmul(w2n_ps[pr],
                                             vsb[su][:, pr * P:(pr + 1) * P],
                                             pn[:],
                                             start=first, stop=last,
                                             skip_group_check=True)

                # drain partials to DRAM + collective
                for pr in range(2):
                    w2nsb = l1.tile([P, 256], F32, name=f"w2nsb{pr}", tag=f"w2nsb{pr}")
                    nc.vector.tensor_copy(w2nsb[:], w2n_ps[pr])
                    nc.sync.dma_start(partial[pr * P:(pr + 1) * P, :], w2nsb[:])
                    ssum = l1.tile([P, 1], F32, name=f"ssum{pr}", tag=f"ssum{pr}")
                    nc.vector.tensor_reduce(ssum[:], scol[pr][:], AX.X, ALU.add)
                    nc.sync.dma_start(
                        partial[256:257, pr * P:(pr + 1) * P], ssum[:])

            nc.gpsimd.collective_compute(
                "AllReduce", ALU.add,
                replica_groups=[[0, 1], [2, 3], [4, 5], [6, 7]],
                ins=[partial[:].opt()], outs=[reduced[:].opt()])

            # ---------------- w2blk prep ----------------
            red = [pp.tile([P, 256], F32, name=f"red{pr}", tag=f"red{pr}") for pr in range(2)]
            sinv = [[pp.tile([KK, 1], F32, name=f"sinv{pr}{hb}",
                             tag=f"sinv{pr}{hb}") for hb in range(2)]
                    for pr in range(2)]
            for pr in range(2):
                nc.sync.dma_start(red[pr][:], reduced[pr * P:(pr + 1) * P, :])
                for hb in range(2):
                    stmp = pp.tile([KK, 1], F32, name=f"stmp{pr}{hb}",
                                   tag=f"stmp{pr}{hb}")
                    off = pr * P + hb * KK
                    nc.sync.dma_start(stmp[:], reduced[256:257, off:off + KK])
                    nc.vector.reciprocal(sinv[pr][hb][:], stmp[:])

            w2blk = [pp.tile([P, P], BF, name=f"w2blk{pr}", tag=f"w2blk{pr}") for pr in range(2)]
            psB_cm = tc.tile_pool(name="psB", bufs=2, space="PSUM")
            psB = psB_cm.__enter__()
            for pr in range(2):
                nc.vector.memset(w2blk[pr][:], 0.0)
                for hb in range(2):
                    rsl = slice(hb * KK, (hb + 1) * KK)
                    csl = slice(pr * P + hb * KK, pr * P + (hb + 1) * KK)
                    tps = psB.tile([KK, KK], F32, name="w2t_ps",
                                   tag="w2t_ps", bufs=2)
                    nc.tensor.matmul(tps[:], red[pr][rsl, csl],
                                     ident[rsl, rsl], is_transpose=True)
                    stg = pp.tile([KK, KK], BF, name=f"w2stg{pr}{hb}",
                                  tag=f"w2stg{pr}{hb}")
                    nc.vector.tensor_scalar(stg[:], tps[:],
                                            sinv[pr][hb][:], None, ALU.mult)
                    nc.sync.dma_start(w2blk[pr][rsl, rsl], stg[:])
            psB_cm.__exit__(None, None, None)

            # ---------------- loop 2 (passes A/B/C) ----------------
            def ln_stats(lp, lps, chunks, sqeng, prec):
                """Compute per-token mean/rsqrt(var) rows for a (2,[P,T])
                chunk pair. Returns (msb, rinvr) SBUF row tiles [1,T]."""
                rdt = F32 if prec else BF
                st = lps.tile([1, T], F32, name="st_ps", tag="st_ps")
                stq = lps.tile([1, T], F32, name="stq_ps", tag="stq_ps")
                for c in range(2):
                    nc.tensor.matmul(st[0:1, :], mcol[:], chunks[c][:],
                                     start=(c == 0), stop=(c == 1))
                for c in range(2):
                    sq = lp.tile([P, T], BF, name="sq", tag="sq")
                    sqeng.tensor_mul(sq[:], chunks[c][:], chunks[c][:])
                    nc.tensor.matmul(stq[0:1, :], mcol[:], sq[:],
                                     start=(c == 0), stop=(c == 1))
                msb = lp.tile([1, T], rdt, name="msb", tag="msb")
                nc.vector.tensor_copy(msb[:], st[0:1, :])
                sqm = lp.tile([1, T], F32, name="sqm", tag="sqm")
                nc.scalar.activation(sqm[:], st[0:1, :], AF.Square)
                varr = lp.tile([1, T], F32, name="varr", tag="varr")
                nc.vector.tensor_tensor(varr[:], stq[0:1, :], sqm[:],
                                        ALU.subtract)
                sdv = lp.tile([1, T], F32, name="sdv", tag="sdv")
                nc.scalar.activation(sdv[:], varr[:], AF.Sqrt,
                                     bias=epsc[0:1, 0:1])
                rinvr = lp.tile([1, T], rdt, name="rinvr", tag="rinvr")
                nc.vector.reciprocal(rinvr[:], sdv[:])
                return msb, rinvr

            def ln_apply(lp, lps, chunks, msb, rinvr, g_sb, be_sb, y_out, prec):
                ones = onesrf if prec else onesr
                mb = lps.tile([P, T], F32, name="mb_ps", tag="mb_ps")
                nc.tensor.matmul(mb[:], ones[:], msb[:])
                rb = lps.tile([P, T], F32, name="rb_ps", tag="rb_ps")
                nc.tensor.matmul(rb[:], ones[:], rinvr[:])
                tdt = F32 if prec else BF
                for c in range(2):
                    t1 = lp.tile([P, T], tdt, name="lnt1", tag="lnt1")
                    nc.vector.tensor_tensor(t1[:], chunks[c][:], mb[:],
                                            ALU.subtract)
                    nc.vector.tensor_tensor(t1[:], t1[:], rb[:], ALU.mult)
                    nc.gpsimd.tensor_scalar(y_out[c][:], t1[:],
                                            g_sb[:, c:c + 1],
                                            be_sb[:, c:c + 1],
                                            ALU.mult, ALU.add)

            ffn_cm = tc.tile_pool(name="ffnp", bufs=2 * NTILES)
            fp_ = ffn_cm.__enter__()
            ffnT = [[None] * 2 for _ in range(NTILES)]

            # ---- pass A: attention out + residual + LN1 ----
            with tc.tile_pool(name="l2a", bufs=3) as l2, \
                 tc.tile_pool(name="l2aps", bufs=1, space="PSUM") as l2ps, \
                 tc.tile_pool(name="stpsA", bufs=2, space="PSUM") as stps:
                for ti in range(NTILES):
                    tsl = slice(ti * T, (ti + 1) * T)
                    Bt = [l2.tile([P, T], BF, name=f"B{pr}", tag=f"B{pr}") for pr in range(2)]
                    for pr in range(2):
                        xp = l2.tile([P, T], BF, name=f"xp{pr}", tag=f"xp{pr}")
                        nc.sync.dma_start(xp[:], xsP_d[pr * P:(pr + 1) * P, tsl])
                        ops = l2ps.tile([P, T], F32, name="o_ps", tag="o_ps")
                        nc.tensor.matmul(ops[:], w2blk[pr][:],
                                         w1uT[ti][pr][:])
                        nc.vector.tensor_tensor(Bt[pr][:], ops[:], xp[:],
                                                ALU.add)
                    msb, rinvr = ln_stats(l2, stps, Bt, nc.gpsimd, False)
                    ffn = [fp_.tile([P, T], BF, name="ffn", tag="ffn")
                           for c in range(2)]
                    ln_apply(l2, l2ps, Bt, msb, rinvr, g1_sb, be1_sb, ffn, False)
                    ffnT[ti] = ffn

            # ---- pass B: FFN up (gelu) + FFN down + residual ----
            r2_cm = tc.tile_pool(name="r2p", bufs=2 * NTILES)
            rp_ = r2_cm.__enter__()
            r2T = [[None] * 2 for _ in range(NTILES)]
            with tc.tile_pool(name="l2b", bufs=3) as l2, \
                 tc.tile_pool(name="l2bps", bufs=1, space="PSUM") as l2ps:
                for ti in range(NTILES):
                    ffn = ffnT[ti]
                    hT = [l2.tile([P, T], BF, name=f"h{m}", tag=f"h{m}") for m in range(4)]
                    for m in range(4):
                        hps = l2ps.tile([P, T], F32, name="h_ps", tag="h_ps")
                        for c in range(2):
                            nc.tensor.matmul(hps[:],
                                             w1_sb[c][:, m * P:(m + 1) * P],
                                             ffn[c][:],
                                             start=(c == 0), stop=(c == 1))
                        nc.scalar.activation(hT[m][:], hps[:], AF.Gelu,
                                             bias=b1_sb[:, m:m + 1])
                    r2 = [rp_.tile([P, T], BF, name="r2", tag="r2")
                          for c in range(2)]
                    for c in range(2):
                        rps = l2ps.tile([P, T], F32, name="r2_ps", tag="r2_ps")
                        for m in range(4):
                            nc.tensor.matmul(rps[:],
                                             w2_sb[m][:, c * P:(c + 1) * P],
                                             hT[m][:],
                                             start=(m == 0), stop=(m == 3))
                        nc.vector.scalar_tensor_tensor(r2[c][:], rps[:],
                                                       b2_sb[:, c:c + 1],
                                                       ffn[c][:],
                                                       ALU.add, ALU.add)
                    r2T[ti] = r2

            # ---- pass C: LN2 + store ----
            with tc.tile_pool(name="l2c", bufs=3) as l2, \
                 tc.tile_pool(name="l2cps", bufs=1, space="PSUM") as l2ps, \
                 tc.tile_pool(name="stpsC", bufs=2, space="PSUM") as stps:
                for ti in range(NTILES):
                    tsl = slice(ti * T, (ti + 1) * T)
                    r2 = r2T[ti]
                    msb, rinvr = ln_stats(l2, stps, r2, nc.gpsimd, True)
                    yt = [l2.tile([P, T], F32, name=f"y{c}", tag=f"y{c}") for c in range(2)]
                    ln_apply(l2, l2ps, r2, msb, rinvr, g2_sb, be2_sb, yt, True)
                    for c in range(2):
                        nc.sync.dma_start(y_d[c * P:(c + 1) * P, tsl], yt[c][:])
            r2_cm.__exit__(None, None, None)
            ffn_cm.__exit__(None, None, None)
            w1up_cm.__exit__(None, None, None)

    lp_cm.__exit__(None, None, None)
    nc.compile()
    return nc


_CACHED = {}

# channel permutation: internal row i = (h, hd) -> reference channel hd*4+h
PERM = np.array([(i % KK) * 4 + i // KK for i in range(DIMV)])


def _hbsel8():
    """[P, 8] lhsT: cols 0-3 for pr=0 (head-block masks in cols 0,1),
    cols 4-7 for pr=1 (masks in cols 2,3); accumulating both matmuls
    yields rows 0-3 = per-head k-rowsums."""
    m = np.zeros((P, 8), np.float32)
    m[0:KK, 0] = 1.0
    m[KK:P, 1] = 1.0
    m[0:KK, 4 + 2] = 1.0
    m[KK:P, 4 + 3] = 1.0
    return m.astype(ml_dtypes.bfloat16)


def _vec2(v, perm=False):
    v = np.asarray(v, np.float32)
    if perm:
        v = v[PERM]
    return np.ascontiguousarray(v.reshape(2, P).T)


def kernel(x, C, G=None, Wq=None, bq=None, Wk=None, bk=None, Wv=None, bv=None,
           W1=None, b1=None, W2=None, b2=None,
           ln1_g=None, ln1_b=None, ln2_g=None, ln2_b=None, **_ignored):
    x = np.ascontiguousarray(np.asarray(x, np.float32))
    C = np.asarray(C, np.float32)
    B, DIM, N = x.shape[0], x.shape[1], x.shape[2]
    assert (B, DIM, N) == (4, 256, 16384), (B, DIM, N)

    if "nc" not in _CACHED:
        _CACHED["nc"] = build_program()
    nc = _CACHED["nc"]

    bf = ml_dtypes.bfloat16
    b16 = lambda a: np.ascontiguousarray(np.asarray(a, np.float32).astype(bf))
    b1v = np.asarray(b1, np.float32)
    shared = dict(
        Wq=b16(Wq), Wk=b16(Wk), Wv=b16(Wv),
        bq=_vec2(bq), bk=_vec2(bk),
        bv=b16(np.asarray(bv, np.float32).reshape(1, DIMV)),
        W1=b16(np.asarray(W1, np.float32)[PERM, :]),
        b1=np.ascontiguousarray(b1v.reshape(4, P).T),
        W2=b16(np.asarray(W2, np.float32)[:, PERM]),
        b2=_vec2(b2, True),
        ln1_g=_vec2(ln1_g, True), ln1_b=_vec2(ln1_b, True),
        ln2_g=_vec2(ln2_g, True), ln2_b=_vec2(ln2_b, True),
        ident=np.eye(P, dtype=np.float32),
        identb=np.eye(P, dtype=np.float32).astype(bf),
        onesr=np.ones((1, P), np.float32).astype(bf),
        onesrf=np.ones((1, P), np.float32),
        mcol=np.full((P, 1), 1.0 / DIMV, np.float32).astype(bf),
        esel=np.repeat(np.eye(4, dtype=np.float32), KK, axis=1).astype(bf),
        hbsel=_hbsel8(),
    )
    xb = x[..., 0].astype(bf)            # (B, DIM, N) bf16
    xbp = np.ascontiguousarray(xb[:, PERM, :])
    in_maps = []
    for c in range(8):
        b, half = c // 2, c % 2
        n0 = half * NT
        m = dict(shared)
        m["xsT"] = np.ascontiguousarray(xb[b, :, n0:n0 + NT])
        m["xsP"] = np.ascontiguousarray(xbp[b, :, n0:n0 + NT])
        m["Cb"] = b16(C[b])
        in_maps.append(m)

    res = run_bass_kernel_spmd(nc, in_maps, list(range(8)), trace=TRACE,
                               **TRACE_KW)
    _CACHED["last_result"] = res

    out = np.empty((B, DIM, N, 1), np.float32)
    for c in range(8):
        b, half = c // 2, c % 2
        n0 = half * NT
        out[b, PERM, n0:n0 + NT, 0] = res.results[c]["y"]
    return out


if __name__ == "__main__":
    nc = build_program()
    print("program built ok")
